# revision 94
# baseline (speedup 1.0000x reference)
"""Trainium2 Bass kernel for nn_MCGraphAttention (edge-scaled multi-head attention).

Reference math (B=4, T=2048, C=256, H=4, D=64):
    x   = nodes * mask
    q,k,v = x @ W{q,k,v}.T            (torch Linear convention)
    s   = (q @ k.T) * H**-0.5         per head
    w   = softmax(s * (3*edge+1))     over keys, edge broadcast over heads
    out = (w @ v, heads merged) @ Wp.T

Mask sparsity (the key win): masked nodes have x=0, so
  - masked KEY j contributes exactly 1 (pre-shift) to the softmax
    denominator and nothing to the numerator (k_j=0 -> arg=0, v_j=0).
  - masked QUERY i yields the uniform-softmax row (1/T)*sum_j v_j @ Wp.T,
    identical for every masked query.
The device therefore only processes the ~1024 UNMASKED rows: the host
gathers them, dropped keys fold into the compile-time constant
C0=(T-KP)*e^-M0 (padded key slots self-count), and the masked-query
output row is read from any padded query column.

Sharding: 8 cores = 4 batches x 2 query-halves. KP=1152 padded keys
(9 chunks), QP=544 padded queries per core (actual max 532).

Device design (per core):
  - scores TRANSPOSED: s[kj, qi] (keys on partitions); softmax-sum falls
    out of AV via a ones column in vN.
  - PSUM plan (8 banks, the scheduling bottleneck): score tiles are
    [128,512] (1 bank) in a 3-deep pool; the 32-wide column tails
    (QP=544=512+32) go to a shared 1-bank slot array; AV accumulates in
    [65,512] tiles (2) plus one tail bank per head. 3 score slots instead
    of 2 eases the mm->multiply->mm slot-recycle serialization.
  - arg=e'*(1.5*q@k) with e'=e+1/3 pre-added on the host: one DVE
    tensor_tensor per iteration for the 512-col A-part plus a 32-col tail
    op (eTails streamed in iteration order; PSUM reads must stay within
    one accumulation-group region).
  - w=exp(arg-20) on ACT in EB=3 batches (bias rides the exp).
  - normalization: den+C0 -> reciprocal -> 64-partition broadcast -> multiply
    evacuating resT. Pass-boundary dances use a DRAM-bounce broadcast whose
    DMA latency hides in the next pass's ramp; the final dances use a
    1-contract PE broadcast matmul (no DMA on the tail critical path).
  - input DMAs consolidated (HWDGE is ~600ns per DMA, serialized).
  - real-HW constraints honored (the cost-model sim is laxer): GPSIMD/Pool
    touches no PSUM (memsets only), tensor ops use at most one PSUM
    operand, matmuls never cross a PSUM bank, vector-op APs stay 2D and
    within one accumulation-group region.
"""

import os
import sys

import numpy as np

for _p in ("/opt/trn_rl_repo",):
    if _p not in sys.path and os.path.isdir(_p):
        sys.path.insert(0, _p)

B, T, C, H = 4, 2048, 256, 4
D = C // H
NCORES = 8
KP = 1152  # padded unmasked-key count (9 chunks of 128)
KC = KP // 128
QP = 544  # padded queries per core (actual qn <= 532; fixed key(0) input)
QA = 512  # A-part column width (one psum bank of f32)
QB = QP - QA  # 32-wide tail
M0 = 20.0  # global softmax shift (safe: args in [-84, 84], row maxes >= 0)
C0 = float((T - KP) * np.exp(-M0))  # denominator constant for dropped keys
EB = 3  # exp batch: iterations per ACT exp op
RR = set()  # reroute experiment: net-negative, Pool TT legality confirmed

# per-pass iteration sequence (kjc, hh): interleaved, hh=0 tail first so its
# normalization overlaps hh=1's last AVs. Same kjc order both passes.
SEQ = [(kjc, hh) for kjc in range(KC - 3) for hh in range(2)]
SEQ += [(KC - 3, 0), (KC - 2, 0), (KC - 1, 0)]
SEQ += [(KC - 3, 1), (KC - 2, 1), (KC - 1, 1)]
KSEQ = [k for k, _ in SEQ]  # tail chunk order for the host-built eTails

_CACHE = {}
_META = {}


def _build_nc(reps=1):
    import concourse.bacc as bacc
    import concourse.mybir as mybir
    import concourse.tile as tile

    f32 = mybir.dt.float32
    f16 = mybir.dt.float16

    nc = bacc.Bacc("TRN2", target_bir_lowering=False, debug=False)

    xT = nc.dram_tensor("xT", [C, KP], f16, kind="ExternalInput").ap()
    xqT = nc.dram_tensor("xqT", [C, QP], f16, kind="ExternalInput").ap()
    eT = nc.dram_tensor("eT", [KP, QA], f16, kind="ExternalInput").ap()
    eTl = nc.dram_tensor("eTl", [128, len(SEQ) * QB], f16, kind="ExternalInput").ap()
    wqT = nc.dram_tensor("wqT", [C, C], f16, kind="ExternalInput").ap()
    wkT = nc.dram_tensor("wkT", [C, C], f16, kind="ExternalInput").ap()
    wvT = nc.dram_tensor("wvT", [C, C], f16, kind="ExternalInput").ap()
    wpT = nc.dram_tensor("wpT", [C, C], f16, kind="ExternalInput").ap()
    bf16 = mybir.dt.bfloat16
    out_t = nc.dram_tensor("out_t", [C, QP], bf16, kind="ExternalOutput").ap()

    with tile.TileContext(nc) as tc:
        for rep in range(reps):
            _emit_rep(nc, tc, rep, xT, xqT, eT, eTl, wqT, wkT, wvT, wpT, out_t)

    nc.compile()
    return nc


def _emit_rep(nc, tc, rep, xT, xqT, eT, eTl, wqT, wkT, wvT, wpT, out_t):
    import concourse.bass as bass
    import concourse.mybir as mybir
    from contextlib import ExitStack

    f32 = mybir.dt.float32
    f16 = mybir.dt.float16
    bf16 = mybir.dt.bfloat16
    ADD = mybir.AluOpType.add
    MULT = mybir.AluOpType.mult
    DIV = mybir.AluOpType.divide
    EXP = mybir.ActivationFunctionType.Exp
    CPY = mybir.ActivationFunctionType.Copy

    HK = KP // 2  # key half per k-projection call
    NIT = 2 * KC  # iterations per hp pass

    rec_scr = nc.dram_tensor(f"rec_scr{rep}", [H, QP], f32).ap()

    with ExitStack() as ctx:
        consts = ctx.enter_context(tc.tile_pool(name=f"consts{rep}", bufs=1))

        # ---- persistent SBUF tensors; DMA order tuned for pipeline start ----
        xT_sb = consts.tile([128, 2 * KP], f16, tag="xT", name="xT_sb")
        xq_sb = consts.tile([128, 2 * QP], f16, tag="xq", name="xq_sb")
        wmap = {}
        for nm in ("wq", "wk", "wv", "wp"):
            wmap[nm] = consts.tile([128, 2 * C], f16, tag=nm, name=f"{nm}_sb")
        wq_sb, wk_sb, wv_sb, wp_sb = wmap["wq"], wmap["wk"], wmap["wv"], wmap["wp"]
        eT_big = consts.tile([128, KC * QA], f16, tag="eT", name="eT_sb")
        eTl_sb = consts.tile([128, NIT * QB], f16, tag="eTl", name="eTl_sb")

        def pview(t, blk):  # [128, 2*blk] tile -> [128, ci, blk] view
            return t.rearrange("p (ci j) -> p ci j", ci=2)

        def dsplit(dram, blk):  # [2*128, blk] dram -> [p, ci, blk] view
            return dram.rearrange("(ci p) j -> p ci j", ci=2)

        eT_src = eT.rearrange("(j p) q -> p j q", j=KC)
        eT_dst = eT_big.rearrange("p (j q) -> p j q", j=KC)

        nc.sync.dma_start(
            out=pview(xT_sb, KP)[:, :, 0:HK], in_=dsplit(xT, KP)[:, :, 0:HK]
        )
        nc.sync.dma_start(out=pview(wk_sb, C), in_=dsplit(wkT, C))
        nc.sync.dma_start(out=pview(xq_sb, QP), in_=dsplit(xqT, QP))
        nc.sync.dma_start(out=pview(wq_sb, C), in_=dsplit(wqT, C))
        nc.sync.dma_start(out=eT_dst[:, 0:2], in_=eT_src[:, 0:2])
        nc.sync.dma_start(out=pview(wv_sb, C), in_=dsplit(wvT, C))
        nc.sync.dma_start(out=eTl_sb, in_=eTl)
        nc.sync.dma_start(
            out=pview(xT_sb, KP)[:, :, HK:KP], in_=dsplit(xT, KP)[:, :, HK:KP]
        )
        nc.sync.dma_start(out=eT_dst[:, 2:5], in_=eT_src[:, 2:5])
        nc.sync.dma_start(out=eT_dst[:, 5:KC], in_=eT_src[:, 5:KC])
        nc.sync.dma_start(out=pview(wp_sb, C), in_=dsplit(wpT, C))

        def wslice(w, ci, co):  # [128, C] ci-block, 128-col co-block
            return w[:, ci * C + co * 128 : ci * C + (co + 1) * 128]

        vN_sb = [
            consts.tile([128, H * (D + 1)], bf16, tag=f"vN{j}", name=f"vN_sb{j}")
            for j in range(KC)
        ]
        qT_sb = [
            consts.tile([128, QP], f16, tag=f"qT{i}", name=f"qT_sb{i}") for i in range(2)
        ]
        kT_sb = [
            consts.tile([128, KP], f16, tag=f"kT{i}", name=f"kT_sb{i}") for i in range(2)
        ]
        resn_sb = [
            consts.tile([128, QP], f16, tag=f"rn{i}", name=f"resn_sb{i}")
            for i in range(2)
        ]
        bias_m0 = consts.tile([128, 1], f32, tag="biasM0", name="bias_m0")
        nc.gpsimd.memset(bias_m0, -M0)
        ones64 = consts.tile([1, 64], f32, tag="ones64", name="ones64")
        nc.gpsimd.memset(ones64, 1.0)
        # preload the Exp activation table while DMAs stream (Copy shares it)
        warm = consts.tile([1, 1], f32, tag="warm", name="warm")
        nc.gpsimd.memset(warm, 0.0)
        nc.scalar.activation(warm, warm, EXP)

        for tch in range(KC):
            # only the ones-columns: the D-blocks are overwritten by proj_v
            ocol = vN_sb[tch].rearrange("p (h e) -> p h e", h=H)[:, :, D : D + 1]
            nc.gpsimd.memset(ocol, 1.0)

        # ---- PSUM plan: spA 3 banks | t64 1 | rtsA 2 | rtsB 2 (1/head) ----
        with (
            tc.tile_pool(name="spA", bufs=3, space="PSUM") as spA,
            tc.tile_pool(name="taux", bufs=1, space="PSUM") as taux,
            tc.tile_pool(name="rpsA", bufs=2, space="PSUM") as rpsA,
            tc.tile_pool(name="raux", bufs=1, space="PSUM") as raux,
            tc.tile_pool(name="small", bufs=4) as small,
            tc.tile_pool(name="wapool", bufs=3) as wapool,
            tc.tile_pool(name="wbpool", bufs=3) as wbpool,
        ):
            t64 = taux.tile([128, 512], f32, tag="t64", name="t64")  # 8 x 64-col slots
            # concurrent accumulation groups need separate banks: one per head
            rtsBb = [
                raux.tile([128, 512], f32, tag=f"rtsB{i}", name=f"rtsB{i}")
                for i in range(2)
            ]
            # PE p-state warm-up during the input-DMA wait (0.65 -> 2.4 GHz
            # needs ~3us of continuous busy); output is never read
            for _ in range(10):
                nc.tensor.matmul(
                    t64[0:64, 384:448], ones64, ones64, start=True, stop=True
                )
            aux_cnt = [0]  # rotates slots 6,7 of t64 for proj/dance tails

            def aux_slot():  # 64-wide scratch, callers slice what they need
                s = aux_cnt[0] % 2
                aux_cnt[0] += 1
                return t64[:, 384 + s * 64 : 448 + s * 64]

            def proj_qk(which, co, half=None, eng_a=None, eng_b=None):
                w_sb, dst = (wq_sb, qT_sb) if which == "q" else (wk_sb, kT_sb)
                W = QP if which == "q" else HK  # k halves stay 576 wide
                ps = spA.tile([128, QA], f32, tag="s", name=f"{which}_ps{co}_{half}")
                tl = aux_slot()[:, 0 : W - QA]
                src_ = pview(xq_sb, QP) if which == "q" else pview(xT_sb, KP)
                off = 0 if which == "q" else half * HK
                for ci in range(2):
                    nc.tensor.matmul(
                        ps,
                        wslice(w_sb, ci, co),
                        src_[:, ci, off : off + QA],
                        start=(ci == 0),
                        stop=(ci == 1),
                    )
                for ci in range(2):
                    nc.tensor.matmul(
                        tl,
                        wslice(w_sb, ci, co),
                        src_[:, ci, off + QA : off + W],
                        start=(ci == 0),
                        stop=(ci == 1),
                    )

                def cp(eng, o, i):
                    if eng is nc.scalar:
                        nc.scalar.copy(o, i)
                    else:
                        eng.tensor_copy(o, i)

                cp(eng_a or nc.scalar, dst[co][:, off : off + QA], ps)
                cp(eng_b or nc.vector, dst[co][:, off + QA : off + W], tl)

            def proj_v(tch, eng=None):
                v_ps = spA.tile([128, QA], f32, tag="s", name=f"v_ps{tch}")
                for ci in range(2):
                    nc.tensor.matmul(
                        v_ps[:, 0:C],
                        pview(xT_sb, KP)[:, ci, tch * 128 : (tch + 1) * 128],
                        wv_sb[:, ci * C : (ci + 1) * C],
                        start=(ci == 0),
                        stop=(ci == 1),
                    )
                v4 = v_ps[:, 0:C].rearrange("p (h d) -> p h d", h=H)
                o4 = vN_sb[tch].rearrange("p (h e) -> p h e", h=H)[:, :, 0:D]
                if eng is nc.scalar:
                    nc.scalar.copy(o4, v4)
                else:
                    nc.vector.tensor_copy(o4, v4)

            proj_qk("k", 0, 0)
            proj_qk("q", 0)
            proj_qk("k", 0, 1)

            it = 0
            pend = []
            pend_av = None  # AV emission for the previous exp batch (sw pipeline)
            pend_dance = []  # hp0 dances, deferred a few its into pass 1
            wa = wb = None


            for hp in range(2):
                rtsA = [
                    rpsA.tile([D + 1, QA], f32, tag="rtsA", name=f"rtsA{hp}_{hh}")
                    for hh in range(2)
                ]
                rtsBs = [rtsBb[hh][:, hp * QB : (hp + 1) * QB] for hh in range(2)]

                # normalization: den+C0, reciprocal, broadcast, multiply. Deferred
                # (pass-boundary) dances run wholly on Pool: ACT is saturated
                # by exps there and DVE carries most STTs; at the tail ACT is
                # idle and the two heads run DVE/Pool in parallel.
                def dance(hh, deferred=False, rtsA=rtsA, rtsBs=rtsBs, hp=hp):
                    h = hp * 2 + hh
                    dst = resn_sb[h // 2]
                    r0 = (h % 2) * 64
                    den = small.tile([1, QP], f32, tag="den", name=f"den{h}")
                    recB = small.tile([64, QP], f32, tag="recB", name=f"recB{h}")
                    if deferred:
                        # pass-boundary: DMA-bounce broadcast; engine-cheap and
                        # the DMA latency hides inside the pass-1 ramp
                        nc.vector.tensor_scalar_add(
                            den[:, 0:QA], rtsA[hh][64:65, :], C0
                        )
                        nc.vector.tensor_scalar_add(
                            den[:, QA:QP], rtsBs[hh][64:65, :], C0
                        )
                        den96 = small.tile(
                            [68, QP // 68], f32, tag="d96", name=f"d96_{h}"
                        )
                        nc.sync.dma_start(out=den96, in_=den)
                        rec96 = small.tile(
                            [68, QP // 68], f32, tag="r96", name=f"r96_{h}"
                        )
                        nc.vector.reciprocal(rec96, den96)
                        nc.sync.dma_start(
                            out=rec_scr[h, :].rearrange("(p x) -> p x", p=68),
                            in_=rec96,
                        )
                        rec_bcast = bass.AP(
                            tensor=rec_scr.tensor,
                            offset=rec_scr.offset + h * QP,
                            ap=[[0, 64], [1, QP]],
                        )
                        nc.sync.dma_start(out=recB, in_=rec_bcast)
                    else:
                        # tail: no DMA latency; ACT (idle here) + PE broadcast,
                        # reciprocal staged through SBUF (one PSUM operand max)
                        nc.scalar.activation(
                            den[:, 0:QA], rtsA[hh][64:65, :], CPY, bias=C0
                        )
                        nc.scalar.activation(
                            den[:, QA:QP], rtsBs[hh][64:65, :], CPY, bias=C0
                        )
                        rec = small.tile([1, QP], f32, tag="rec", name=f"rec{h}")
                        nc.vector.reciprocal(rec, den)
                        denA = spA.tile([128, QA], f32, tag="s", name=f"denA{h}")
                        denB = aux_slot()[:, 0:QB]
                        nc.tensor.matmul(
                            denA[0:64, :], ones64, rec[:, 0:QA], start=True, stop=True
                        )
                        nc.tensor.matmul(
                            denB[0:64, :], ones64, rec[:, QA:QP], start=True, stop=True
                        )
                        nc.scalar.copy(recB[:, 0:QA], denA[0:64, :])
                        nc.scalar.copy(recB[:, QA:QP], denB[0:64, :])
                    nc.vector.tensor_tensor(
                        out=dst[r0 : r0 + 64, 0:QA],
                        in0=rtsA[hh][0:64, :],
                        in1=recB[:, 0:QA],
                        op=MULT,
                    )
                    nc.vector.tensor_tensor(
                        out=dst[r0 : r0 + 64, QA:QP],
                        in0=rtsBs[hh][0:64, :],
                        in1=recB[:, QA:QP],
                        op=MULT,
                    )

                def make_av(batch, rtsA=rtsA, rtsBs=rtsBs, hp=hp):
                    def emit_av():
                        for phh, pkjc, psl, pwb in batch:
                            lhsT = vN_sb[pkjc][
                                :,
                                (hp * 2 + phh) * (D + 1) : (hp * 2 + phh + 1) * (D + 1),
                            ]
                            nc.tensor.matmul(
                                rtsA[phh],
                                lhsT,
                                pwb[:, psl * QA : (psl + 1) * QA],
                                start=(pkjc == 0),
                                stop=(pkjc == KC - 1),
                            )
                            nc.tensor.matmul(
                                rtsBs[phh][0 : D + 1, :],
                                lhsT,
                                pwb[:, EB * QA + psl * QB : EB * QA + (psl + 1) * QB],
                                start=(pkjc == 0),
                                stop=(pkjc == KC - 1),
                            )
                    return emit_av

                for kjc, hh in SEQ:
                        h = hp * 2 + hh
                        co, row = h // 2, (h % 2) * 64
                        sp = spA.tile([128, QA], f32, tag="s", name=f"sp{it}")
                        tl = t64[:, (it % 6) * QB : (it % 6 + 1) * QB]
                        nc.tensor.matmul(
                            sp,
                            kT_sb[co][row : row + 64, kjc * 128 : (kjc + 1) * 128],
                            qT_sb[co][row : row + 64, 0:QA],
                            start=True,
                            stop=True,
                        )
                        nc.tensor.matmul(
                            tl,
                            kT_sb[co][row : row + 64, kjc * 128 : (kjc + 1) * 128],
                            qT_sb[co][row : row + 64, QA:QP],
                            start=True,
                            stop=True,
                        )
                        eb = EB
                        slot = it % EB
                        if slot == 0:
                            wa = wapool.tile([128, eb * QP], f32, tag=f"warg{eb}", name=f"wa{it}")
                            wb = wbpool.tile([128, eb * QP], bf16, tag=f"wexp{eb}", name=f"wb{it}")
                        # e' = e+1/3 is pre-added on the host: the multiply
                        # is a plain tensor_tensor. Rerouted pass-0 iterations
                        # go ACT-evac -> Pool (SBUF-only), easing DVE.
                        if it in RR:
                            s16 = small.tile([128, QA], f16, tag="s16", name=f"s16_{it}")
                            nc.scalar.copy(s16, sp)
                            nc.gpsimd.tensor_tensor(
                                out=wa[:, slot * QA : (slot + 1) * QA],
                                in0=eT_big[:, kjc * QA : (kjc + 1) * QA],
                                in1=s16,
                                op=MULT,
                            )
                        else:
                            nc.vector.tensor_tensor(
                                out=wa[:, slot * QA : (slot + 1) * QA],
                                in0=eT_big[:, kjc * QA : (kjc + 1) * QA],
                                in1=sp,
                                op=MULT,
                            )
                        # per-iteration tail: psum reads stay inside one
                        # accumulation-group region on real HW
                        nc.vector.tensor_tensor(
                            out=wa[:, eb * QA + slot * QB : eb * QA + (slot + 1) * QB],
                            in0=eTl_sb[:, (it % NIT) * QB : (it % NIT + 1) * QB],
                            in1=tl,
                            op=MULT,
                        )
                        pend.append((hh, kjc, slot, wb))
                        if slot == eb - 1:
                            nc.scalar.activation(wb, wa, EXP, bias=bias_m0)
                            if pend_av is not None:
                                pend_av()
                            pend_av = make_av(pend)
                            pend = []
                        if 3 <= it < KC + 3:
                            proj_v(it - 3, eng=nc.scalar)
                        if hp == 0:  # stage heads {2,3} projections late in pass 0
                            if it == 13:
                                proj_qk("q", 1, eng_a=nc.scalar, eng_b=nc.vector)
                            elif it == 15:
                                proj_qk("k", 1, 0, eng_a=nc.scalar, eng_b=nc.vector)
                            elif it == 17:
                                proj_qk("k", 1, 1, eng_a=nc.scalar, eng_b=nc.vector)
                        elif it == NIT + 1 and pend_dance:
                            pend_dance[0]()  # hp0 dance(0): AV(b4) done
                        elif it == NIT + 3 and pend_dance:
                            pend_dance[1]()  # hp0 dance(1): AV(b5) emitted at it 20
                            pend_dance = []
                        it += 1
                if hp == 0:
                    # pend_av (last hh1 batch) rides into pass 1; dances deferred
                    pend_dance = [
                        lambda d=dance: d(0, deferred=True),
                        lambda d=dance: d(1, deferred=True),
                    ]
                else:
                    dance(0)  # hh0's AVs all emitted (staggered tail)
                    if pend_av is not None:
                        pend_av()
                        pend_av = None
                    dance(1)

            # ---- output projection inside the main pools (no close drains);
            # A-part in a spA tile, tail in an aux slot ----
            for co in range(2):
                oA = spA.tile([128, QA], f32, tag="s", name=f"oA{co}")
                oB = aux_slot()[:, 0:QB]
                for ci in range(2):
                    nc.tensor.matmul(
                        oA,
                        wslice(wp_sb, ci, co),
                        resn_sb[ci][:, 0:QA],
                        start=(ci == 0),
                        stop=(ci == 1),
                    )
                for ci in range(2):
                    nc.tensor.matmul(
                        oB,
                        wslice(wp_sb, ci, co),
                        resn_sb[ci][:, QA:QP],
                        start=(ci == 0),
                        stop=(ci == 1),
                    )
                outsb = consts.tile([128, QP], bf16, tag=f"outsb{co}", name=f"outsb{co}")
                if co == 0:
                    nc.scalar.copy(outsb[:, 0:QA], oA)  # ACT idle at the tail
                    nc.vector.tensor_copy(outsb[:, QA:QP], oB)
                else:
                    nc.vector.tensor_copy(outsb[:, 0:QA], oA)
                    nc.vector.tensor_copy(outsb[:, QA:QP], oB)
                nc.sync.dma_start(out=out_t[co * 128 : (co + 1) * 128, :], in_=outsb)

def get_nc():
    if "nc" not in _CACHE:
        _CACHE["nc"] = _build_nc()
    return _CACHE["nc"]


def make_in_maps(**inputs):
    nodes = np.asarray(inputs["nodes"], np.float32)
    edge = np.asarray(inputs["edge_index"], np.float32)
    mask = np.asarray(inputs["mask"])
    Wq = np.asarray(inputs["Wq"], np.float32)
    Wk = np.asarray(inputs["Wk"], np.float32)
    Wv = np.asarray(inputs["Wv"], np.float32)
    Wp = np.asarray(inputs["Wp"], np.float32)

    wq_t = np.ascontiguousarray((3.0 * H**-0.5) * Wq.T).astype(np.float16)
    wk_t = np.ascontiguousarray(Wk.T).astype(np.float16)
    wv_t = np.ascontiguousarray(Wv.T).astype(np.float16)
    wp_t = np.ascontiguousarray(Wp.T).astype(np.float16)

    in_maps = []
    meta = []
    for b in range(B):
        kidx = np.flatnonzero(mask[b])
        n = len(kidx)
        assert n <= KP, f"batch {b}: {n} unmasked keys > KP={KP}"
        xk = np.zeros((KP, C), np.float32)
        xk[:n] = nodes[b][kidx]
        xkT = np.ascontiguousarray(xk.T).astype(np.float16)
        h1 = (n + 1) // 2
        for qh, qidx in enumerate((kidx[:h1], kidx[h1:])):
            qn = len(qidx)
            assert qn < QP, f"batch {b} half {qh}: {qn} queries >= QP={QP}"
            xq = np.zeros((QP, C), np.float32)
            xq[:qn] = nodes[b][qidx]
            # e' = e + 1/3 pre-added on the host: the device multiply is a
            # plain tensor_tensor (padded slots get 1/3, harmless: s=0 there)
            eTc = np.full((KP, QP), np.float16(1.0 / 3.0))
            eTc[:n, :qn] = (edge[b][np.ix_(qidx, kidx)].T + 1.0 / 3.0).astype(
                np.float16
            )
            # tails in device iteration order (kjc per SEQ), both passes share
            eTl = np.concatenate(
                [eTc[k * 128 : (k + 1) * 128, QA:QP] for k in KSEQ], axis=1
            )
            in_maps.append(
                {
                    "xT": xkT,
                    "xqT": np.ascontiguousarray(xq.T).astype(np.float16),
                    "eT": np.ascontiguousarray(eTc[:, 0:QA]),
                    "eTl": np.ascontiguousarray(eTl),
                    "wqT": wq_t,
                    "wkT": wk_t,
                    "wvT": wv_t,
                    "wpT": wp_t,
                }
            )
            meta.append((b, qidx))
    _META["meta"] = meta
    _META["mask"] = mask
    return in_maps


def assemble(results):
    meta, mask = _META["meta"], _META["mask"]
    out = np.empty((B, T, C), np.float32)
    for c, (b, qidx) in enumerate(meta):
        res = np.asarray(results[c]["out_t"], np.float32)  # [C, QP] (bf16 on device)
        out[b, qidx, :] = res[:, : len(qidx)].T
        if c % 2 == 0:  # the batch's first core supplies the masked-row value
            out[b, ~mask[b], :] = res[:, QP - 1]
    return out


def run(in_maps, trace=False):
    from concourse.bass_utils import run_bass_kernel_spmd

    nc = get_nc()
    if trace:
        try:
            return run_bass_kernel_spmd(nc, in_maps, list(range(NCORES)), trace=True)
        except (ImportError, ModuleNotFoundError):
            pass  # NTFF hook unavailable in this environment
    return run_bass_kernel_spmd(nc, in_maps, list(range(NCORES)), trace=False)


def kernel(**inputs):
    res = run(make_in_maps(**inputs), trace=False)
    return assemble(res.results)
